# revision 1
# baseline (speedup 1.0000x reference)
"""Trainium2 Bass kernel for nn_Fields: 16 per-field MLPs (3->16->16->3, ReLU)
applied to 1M points, sharded over 8 NeuronCores along the point axis.

Dataflow per core (points sharded N/8, padded to slabs of 4096 = 8 chunks of
512), processed slab-at-a-time in three layer phases so PE matmuls stream
against PSUM->SBUF evacuation with only 4 rotating PSUM slots:
  DMA x (+ones row) -> L1 matmuls (K=4, M=128) -> PSUM f32
  -> ReLU evac (DVE/ScalarE, greedy-balanced) -> SBUF bf16 h1
  -> L2 block-diag matmuls (K=128) -> PSUM -> ReLU+b2 evac -> SBUF bf16 h2
  -> L3 col-tiled matmuls (M=24 at col groups 0/32/64/96) -> one PSUM bank
  -> Identity+b3 evac -> SBUF bf16 -> one slab DMA to a quarter-structured
  [128, n_pad/2] bf16 DRAM tensor (gap rows traded for 4x fewer DMAs);
  the host upcasts and unscrambles quarters into [16, 3, N] f32.
b1 is folded into L1 via a ones row appended to x; b2/b3 ride the evacuation
ops as per-partition bias vectors. All evac destinations are bf16 so DVE runs
in its 2x_1P mode; the DVE/ScalarE split uses silicon-calibrated costs.
"""

from contextlib import ExitStack

import ml_dtypes
import numpy as np

import concourse.bass as bass
import concourse.mybir as mybir
import concourse.tile as tile
from concourse import bacc
from concourse.bass_utils import run_bass_kernel_spmd

N_CORES = 8
NFIELDS = 16
HID = 16
C = 512  # chunk = one fp32 PSUM bank of matmul output
PAIR = 2 * C  # 1024 points
GROUP_PAIRS = 4
GROUP = GROUP_PAIRS * PAIR  # 4096 points

BF16 = mybir.dt.bfloat16
F32 = mybir.dt.float32
AF = mybir.ActivationFunctionType
ALU = mybir.AluOpType

_cache = {}


def build(n_pad, iters=1, evac_mode="balanced"):
    """Build the per-core Bass program for n_pad points (multiple of GROUP)."""
    assert n_pad % GROUP == 0
    SLAB = GROUP  # slab: 4 pairs = 4096 points (8192 measured worse)
    group_sizes = [SLAB] * (n_pad // SLAB)
    if n_pad % SLAB:
        group_sizes.append(n_pad % SLAB)

    nc = bacc.Bacc(None, target_bir_lowering=False)
    xq = nc.declare_dram_parameter("xq", [4, n_pad], BF16, isOutput=False)
    w1s_d = nc.declare_dram_parameter("w1s", [4, 256], BF16, isOutput=False)
    w2s_d = nc.declare_dram_parameter("w2s", [128, 256], BF16, isOutput=False)
    w3s_d = nc.declare_dram_parameter("w3s", [128, 48], BF16, isOutput=False)
    b2v_d = nc.declare_dram_parameter("b2v", [128, 2], F32, isOutput=False)
    b3v_d = nc.declare_dram_parameter("b3v", [128, 1], F32, isOutput=False)
    y = nc.declare_dram_parameter("y", [128, n_pad // 2], BF16, isOutput=True)

    with ExitStack() as ctx:
        tc = ctx.enter_context(tile.TileContext(nc))
        consts = ctx.enter_context(tc.tile_pool(name="consts", bufs=1))
        xpool = ctx.enter_context(tc.tile_pool(name="xpool", bufs=2))
        h1pool = ctx.enter_context(tc.tile_pool(name="h1pool", bufs=12))
        h2pool = ctx.enter_context(tc.tile_pool(name="h2pool", bufs=12))
        opool = ctx.enter_context(tc.tile_pool(name="opool", bufs=2))
        pspool = ctx.enter_context(tc.tile_pool(name="pspool", bufs=4, space="PSUM"))
        
        w1s = consts.tile([4, 256], BF16)
        nc.sync.dma_start(out=w1s, in_=w1s_d[:, :])
        w2s = consts.tile([128, 256], BF16)
        nc.sync.dma_start(out=w2s, in_=w2s_d[:, :])
        w3s = consts.tile([128, 48], BF16)
        nc.sync.dma_start(out=w3s, in_=w3s_d[:, :])
        b2v = consts.tile([128, 2], F32)
        nc.sync.dma_start(out=b2v, in_=b2v_d[:, :])
        b3v = consts.tile([128, 1], F32)
        nc.sync.dma_start(out=b3v, in_=b3v_d[:, :])

        # Greedy engine balancing for PSUM->SBUF evacuation ops: ScalarE
        # (Activation) runs 1 elem/cyc @1.2GHz with 172cyc PSUM const; DVE
        # 1 elem/cyc @0.96GHz with 120cyc const. Assign each op to the
        # engine with lower accumulated busy time.
        load = {"act": 0.0, "dve": 0.0}

        def evac(out_ap, in_ap, fd, bias=None, relu=True):
            # HW-calibrated (A/B on silicon): DVE gets 2x_1P on bf16 dst
            # even from fp32 PSUM; ScalarE runs ~1.45x its table.
            cost_act = (250 + 1.45 * fd) / 1.2
            cost_dve = (120 + fd / 2) / 0.96
            if evac_mode == "act":
                cost_dve = 1e18
            elif evac_mode == "dve":
                cost_act = 1e18
            if load["act"] + cost_act <= load["dve"] + cost_dve:
                load["act"] += cost_act
                if relu:
                    nc.scalar.activation(out_ap, in_ap, AF.Relu,
                                         bias=0.0 if bias is None else bias)
                else:
                    nc.scalar.activation(out_ap, in_ap, AF.Identity,
                                         bias=0.0 if bias is None else bias)
            else:
                load["dve"] += cost_dve
                if relu:
                    if bias is None:
                        nc.vector.tensor_scalar_max(out_ap, in_ap, 0.0)
                    else:
                        nc.vector.tensor_scalar(out_ap, in_ap, bias, 0.0,
                                                ALU.add, ALU.max)
                else:
                    if bias is None:
                        nc.vector.tensor_copy(out_ap, in_ap)
                    else:
                        nc.vector.tensor_scalar_add(out_ap, in_ap, bias)

        def body(_=None):
            gc = 0
            for g, gsz in enumerate(group_sizes):
                npairs = gsz // PAIR
                NCH = 2 * npairs
                xsb = xpool.tile([4, GROUP], BF16, tag="xsb",
                                 name=f"xsb_{g}")
                nc.gpsimd.dma_start(out=xsb[0:4, 0:gsz],
                                    in_=xq[0:4, gc : gc + gsz])

                # ---- phase L1: all chunks of the slab ----
                h1sb = []
                h1ps = []
                for c in range(NCH):
                    h1ps.append(pspool.tile([128, 2 * C], F32, tag="hps", name=f"h1ps_{g}_{c}"))
                    for half in range(2):
                        nc.tensor.matmul(
                            h1ps[c][:, half * C : half * C + C],
                            w1s[0:4, 128 * half : 128 * half + 128],
                            xsb[0:4, c * C : c * C + C],
                            start=True, stop=True,
                        )
                    t = h1pool.tile([128, 2 * C], BF16, tag="h1sb",
                                    name=f"h1sb_{g}_{c}")
                    evac(t, h1ps[c], 2 * C)
                    h1sb.append(t)

                # ---- phase L2: all half-a matmuls, then all half-b ----
                h2A, h2B = [], []
                for p in range(npairs):
                    t = pspool.tile([128, 2 * C], F32, tag="hps",
                                    name=f"h2a_{g}_{p}")
                    nc.tensor.matmul(t[:, 0:C], w2s[:, 0:128],
                                     h1sb[2 * p][:, 0:C], start=True, stop=True)
                    nc.tensor.matmul(t[:, C : 2 * C], w2s[:, 0:128],
                                     h1sb[2 * p + 1][:, 0:C],
                                     start=True, stop=True)
                    h2A.append(t)
                h2Asb, h2Bsb = [], []
                for p in range(npairs):
                    t = h2pool.tile([128, 2 * C], BF16, tag="h2sb",
                                    name=f"h2asb_{g}_{p}")
                    evac(t, h2A[p], 2 * C, bias=b2v[:, 0:1])
                    h2Asb.append(t)
                for p in range(npairs):
                    t = pspool.tile([128, 2 * C], F32, tag="hps",
                                    name=f"h2b_{g}_{p}")
                    nc.tensor.matmul(t[:, 0:C], w2s[:, 128:256],
                                     h1sb[2 * p][:, C : 2 * C],
                                     start=True, stop=True)
                    nc.tensor.matmul(t[:, C : 2 * C], w2s[:, 128:256],
                                     h1sb[2 * p + 1][:, C : 2 * C],
                                     start=True, stop=True)
                    h2B.append(t)
                for p in range(npairs):
                    t = h2pool.tile([128, 2 * C], BF16, tag="h2sb",
                                    name=f"h2bsb_{g}_{p}")
                    evac(t, h2B[p], 2 * C, bias=b2v[:, 1:2])
                    h2Bsb.append(t)

                # ---- phase L3: 4 col-tiled matmuls per pair into one bank ----
                outsb = opool.tile([128, GROUP // 2], BF16, tag="outsb",
                                   name=f"outsb_{g}")
                pb = (gc // PAIR) * C  # y col base = global pair index * C
                gc += gsz
                for p in range(npairs):
                    oph = pspool.tile([128, C], F32, tag="hps", name=f"oph_{g}_{p}")
                    nc.tensor.matmul(oph[0:24, :], w3s[:, 0:24],
                                     h2Asb[p][:, 0:C], start=True, stop=True,
                                     tile_position=(0, 0))
                    nc.tensor.matmul(oph[64:88, :], w3s[:, 0:24],
                                     h2Asb[p][:, C : 2 * C], start=True,
                                     stop=True, tile_position=(0, 64))
                    nc.tensor.matmul(oph[32:56, :], w3s[:, 24:48],
                                     h2Bsb[p][:, 0:C], start=True, stop=True,
                                     tile_position=(0, 32))
                    nc.tensor.matmul(oph[96:120, :], w3s[:, 24:48],
                                     h2Bsb[p][:, C : 2 * C], start=True,
                                     stop=True, tile_position=(0, 96))
                    evac(outsb[0:120, p * C : p * C + C], oph[0:120, :], C,
                         bias=b3v[0:120, 0:1], relu=False)
                nc.sync.dma_start(
                    out=y[0:120, pb : pb + npairs * C],
                    in_=outsb[0:120, 0 : npairs * C])

        if iters == 1:
            body()
        else:
            with tc.For_i(0, iters, 1):
                body()
    nc.finalize()
    return nc


def prep_weights(W1, b1, W2, b2, W3, b3):
    W1 = np.asarray(W1, np.float32); b1 = np.asarray(b1, np.float32)
    W2 = np.asarray(W2, np.float32); b2 = np.asarray(b2, np.float32)
    W3 = np.asarray(W3, np.float32); b3 = np.asarray(b3, np.float32)
    w1s = np.zeros((4, 256), np.float32)
    for half in range(2):
        fb = 8 * half
        for fl in range(8):
            for h in range(HID):
                w1s[0:3, 128 * half + 16 * fl + h] = W1[fb + fl, h, :]
                w1s[3, 128 * half + 16 * fl + h] = b1[fb + fl, h]
    w2s = np.zeros((128, 256), np.float32)
    for half in range(2):
        fb = 8 * half
        for fl in range(8):
            blk = W2[fb + fl]  # [g2, h]
            w2s[16 * fl : 16 * fl + 16,
                128 * half + 16 * fl : 128 * half + 16 * fl + 16] = blk.T
    w3s = np.zeros((128, 48), np.float32)
    for half in range(2):
        fb = 8 * half
        for fl in range(8):
            blk = W3[fb + fl]  # [o, h]
            w3s[16 * fl : 16 * fl + 16,
                24 * half + 3 * fl : 24 * half + 3 * fl + 3] = blk.T
    b2v = np.zeros((128, 2), np.float32)
    for half in range(2):
        b2v[:, half] = b2[8 * half : 8 * half + 8].reshape(128)
    b3v = np.zeros((128, 1), np.float32)
    for q in range(4):
        fb = 8 * (q % 2)
        b3v[32 * q : 32 * q + 24, 0] = b3[fb : fb + 8].reshape(24)
    bf = ml_dtypes.bfloat16
    return {
        "w1s": w1s.astype(bf), "w2s": w2s.astype(bf), "w3s": w3s.astype(bf),
        "b2v": b2v, "b3v": b3v,
    }


def _get_nc(n_pad, iters=1, evac_mode="balanced"):
    key = (n_pad, iters, evac_mode)
    if key not in _cache:
        _cache[key] = build(n_pad, iters, evac_mode)
    return _cache[key]


def run(x_np, weights, n_pad, iters=1, n=None):
    """x_np: [3, N] f32 full; returns [16, 3, N] f32."""
    if n is None:
        n = x_np.shape[1]
    assert n % N_CORES == 0
    npc = n // N_CORES
    assert npc <= n_pad
    nc = _get_nc(n_pad, iters)
    bf = ml_dtypes.bfloat16
    in_maps = []
    for c in range(N_CORES):
        xs = np.zeros((4, n_pad), np.float32)
        xs[0:3, :npc] = x_np[:, c * npc : (c + 1) * npc]
        xs[3, :] = 1.0
        in_maps.append({"xq": xs.astype(bf), **weights})
    res = run_bass_kernel_spmd(nc, in_maps, core_ids=list(range(N_CORES)))
    out = np.empty((NFIELDS, 3, n), np.float32)
    npr = n_pad // PAIR
    for c in range(N_CORES):
        yc = np.asarray(res.results[c]["y"], np.float32)  # [128, n_pad//2]
        yv = yc.reshape(128, npr, C)
        oc = np.empty((NFIELDS, 3, npr, 2, C), np.float32)
        for q in range(4):
            blk = yv[32 * q : 32 * q + 24].reshape(8, 3, npr, C)
            oc[8 * (q % 2) : 8 * (q % 2) + 8, :, :, q // 2, :] = blk
        out[:, :, c * npc : (c + 1) * npc] = \
            oc.reshape(NFIELDS, 3, n_pad)[:, :, :npc]
    return out


def kernel(x, W1, b1, W2, b2, W3, b3, D):
    x = np.asarray(x, np.float32)
    n = x.shape[2]
    npc = n // N_CORES
    n_pad = ((npc + GROUP - 1) // GROUP) * GROUP
    weights = prep_weights(W1, b1, W2, b2, W3, b3)
    return run(x[0], weights, n_pad)



# revision 2
# speedup vs baseline: 1.0357x; 1.0357x over previous
"""Trainium2 Bass kernel for nn_Fields: 16 per-field MLPs (3->16->16->3, ReLU)
applied to 1M points, sharded over 8 NeuronCores along the point axis.

Dataflow per core (points sharded N/8, padded to slabs of 4096 = 8 chunks of
512), processed pair-of-chunks at a time in three layer phases:
  L1: 4-way ROW-TILED concurrent matmuls (K=4 at row groups 0/32/64/96,
      x replicated at those partition stripes) -> one ~213ns PE span computes
      h1 for BOTH chunks of the pair (silicon-measured 3.1x vs serial).
  L2: block-diag [128x128] per field-half, split as 2 col-tiles of M=64 so
      the two matmuls stream concurrently and weight loads shrink.
  L3: 4-way COL-TILED (M=24 at col groups 0/32/64/96) into one PSUM bank.
Evacuation (the true bottleneck: DVE and ACT both run ~1 elem/cyc/lane from
fp32 PSUM -- 2x modes need 2-byte sources, so PSUM reads can't accelerate):
relu h1 -> DVE (1-ALU tensor_scalar_max), relu+b2 h2 -> ACT (vector bias is
free there), y+b3 -> DVE, balanced by silicon-measured costs.
b1 is folded into L1 via a ones row in x. Output rides a quarter-structured
[128, n_pad/2] bf16 DRAM tensor; the host upcasts and unscrambles.
"""

from contextlib import ExitStack

import ml_dtypes
import numpy as np

import concourse.bass as bass
import concourse.mybir as mybir
import concourse.tile as tile
from concourse import bacc
from concourse.bass_utils import run_bass_kernel_spmd

N_CORES = 8
NFIELDS = 16
HID = 16
C = 512  # chunk = one fp32 PSUM bank of matmul output
PAIR = 2 * C  # 1024 points
GROUP = 4096  # slab

BF16 = mybir.dt.bfloat16
F32 = mybir.dt.float32
AF = mybir.ActivationFunctionType
ALU = mybir.AluOpType

_cache = {}


def build(n_pad, iters=1):
    """Build the per-core Bass program for n_pad points (multiple of GROUP)."""
    assert n_pad % GROUP == 0
    n_slabs = n_pad // GROUP

    nc = bacc.Bacc(None, target_bir_lowering=False)
    xq = nc.declare_dram_parameter("xq", [4, n_pad], BF16, isOutput=False)
    w1q_d = nc.declare_dram_parameter("w1q", [128, 128], BF16, isOutput=False)
    w2s_d = nc.declare_dram_parameter("w2s", [128, 256], BF16, isOutput=False)
    w3s_d = nc.declare_dram_parameter("w3s", [128, 48], BF16, isOutput=False)
    b2v_d = nc.declare_dram_parameter("b2v", [128, 2], F32, isOutput=False)
    b3v_d = nc.declare_dram_parameter("b3v", [128, 1], F32, isOutput=False)
    y = nc.declare_dram_parameter("y", [128, n_pad // 2], BF16, isOutput=True)

    with ExitStack() as ctx:
        tc = ctx.enter_context(tile.TileContext(nc))
        consts = ctx.enter_context(tc.tile_pool(name="consts", bufs=1))
        xpool = ctx.enter_context(tc.tile_pool(name="xpool", bufs=2))
        h1pool = ctx.enter_context(tc.tile_pool(name="h1pool", bufs=6))
        h2pool = ctx.enter_context(tc.tile_pool(name="h2pool", bufs=6))
        opool = ctx.enter_context(tc.tile_pool(name="opool", bufs=2))
        pspool = ctx.enter_context(tc.tile_pool(name="pspool", bufs=4, space="PSUM"))

        w1q = consts.tile([128, 128], BF16)
        nc.sync.dma_start(out=w1q, in_=w1q_d[:, :])
        w2s = consts.tile([128, 256], BF16)
        nc.sync.dma_start(out=w2s, in_=w2s_d[:, :])
        w3s = consts.tile([128, 48], BF16)
        nc.sync.dma_start(out=w3s, in_=w3s_d[:, :])
        b2v = consts.tile([128, 2], F32)
        nc.sync.dma_start(out=b2v, in_=b2v_d[:, :])
        b3v = consts.tile([128, 1], F32)
        nc.sync.dma_start(out=b3v, in_=b3v_d[:, :])

        # Greedy engine balancing with silicon-measured evac costs (ns):
        #   ACT: 288 + 1.086*fd (per-partition bias vector costs nothing)
        #   DVE: 125 + 1.12*fd (1-ALU) / 125 + 1.28*fd (2-ALU with bias)
        load = {"act": 0.0, "dve": 0.0}

        def evac(out_ap, in_ap, fd, bias=None, relu=True):
            cost_act = 288 + 1.086 * fd
            cost_dve = 125 + (1.28 if bias is not None else 1.12) * fd
            if load["act"] + cost_act <= load["dve"] + cost_dve:
                load["act"] += cost_act
                nc.scalar.activation(out_ap, in_ap,
                                     AF.Relu if relu else AF.Identity,
                                     bias=0.0 if bias is None else bias)
            else:
                load["dve"] += cost_dve
                if relu:
                    if bias is None:
                        nc.vector.tensor_scalar_max(out_ap, in_ap, 0.0)
                    else:
                        nc.vector.tensor_scalar(out_ap, in_ap, bias, 0.0,
                                                ALU.add, ALU.max)
                else:
                    if bias is None:
                        nc.vector.tensor_copy(out_ap, in_ap)
                    else:
                        nc.vector.tensor_scalar_add(out_ap, in_ap, bias)

        def body(_=None):
            for g in range(n_slabs):
                goff = g * GROUP
                xsb = xpool.tile([128, GROUP], BF16, tag="xsb", name=f"xsb_{g}")
                for s in range(4):
                    nc.gpsimd.dma_start(out=xsb[32 * s:32 * s + 4, :],
                                        in_=xq[0:4, goff:goff + GROUP])

                outsb = opool.tile([128, GROUP // 2], BF16, tag="outsb",
                                   name=f"outsb_{g}")
                for p in range(GROUP // PAIR):
                    c0 = p * PAIR          # chunk offsets within slab
                    c1 = c0 + C
                    # ---- L1: 4 concurrent row-tiled matmuls ----
                    h1c0 = pspool.tile([128, PAIR], F32, tag="hps",
                                       name=f"h1c0_{g}_{p}")
                    h1c1 = pspool.tile([128, PAIR], F32, tag="hps",
                                       name=f"h1c1_{g}_{p}")
                    for s, (dst, col) in enumerate(
                            ((h1c0, c0), (h1c0, c0), (h1c1, c1), (h1c1, c1))):
                        half = s % 2
                        nc.tensor.matmul(
                            dst[:, half * C:half * C + C],
                            w1q[32 * s:32 * s + 4, 0:128],
                            xsb[32 * s:32 * s + 4, col:col + C],
                            start=True, stop=True,
                            tile_position=(32 * s, 0))
                    h1sb = h1pool.tile([128, 2 * PAIR], BF16, tag="h1sb",
                                       name=f"h1sb_{g}_{p}")
                    evac(h1sb[:, 0:PAIR], h1c0, PAIR)
                    evac(h1sb[:, PAIR:2 * PAIR], h1c1, PAIR)

                    # ---- L2: per half, 2 chunks x 2 col-tiles of M=64 ----
                    h2A = pspool.tile([128, PAIR], F32, tag="hps",
                                      name=f"h2a_{g}_{p}")
                    h2B = pspool.tile([128, PAIR], F32, tag="hps",
                                      name=f"h2b_{g}_{p}")
                    for ci in range(2):
                        rhsA = h1sb[:, ci * PAIR:ci * PAIR + C]
                        for j in range(2):
                            nc.tensor.matmul(
                                h2A[64 * j:64 * j + 64, ci * C:ci * C + C],
                                w2s[:, 64 * j:64 * j + 64], rhsA,
                                start=True, stop=True,
                                tile_position=(0, 64 * j))
                    for ci in range(2):
                        rhsB = h1sb[:, ci * PAIR + C:ci * PAIR + 2 * C]
                        for j in range(2):
                            nc.tensor.matmul(
                                h2B[64 * j:64 * j + 64, ci * C:ci * C + C],
                                w2s[:, 128 + 64 * j:128 + 64 * j + 64], rhsB,
                                start=True, stop=True,
                                tile_position=(0, 64 * j))
                    h2Asb = h2pool.tile([128, PAIR], BF16, tag="h2sb",
                                        name=f"h2asb_{g}_{p}")
                    evac(h2Asb, h2A, PAIR, bias=b2v[:, 0:1])
                    h2Bsb = h2pool.tile([128, PAIR], BF16, tag="h2sb",
                                        name=f"h2bsb_{g}_{p}")
                    evac(h2Bsb, h2B, PAIR, bias=b2v[:, 1:2])

                    # ---- L3: 4 col-tiled matmuls into one PSUM bank ----
                    oph = pspool.tile([128, PAIR], F32, tag="hps",
                                      name=f"oph_{g}_{p}")
                    nc.tensor.matmul(oph[0:24, 0:C], w3s[:, 0:24],
                                     h2Asb[:, 0:C], start=True, stop=True,
                                     tile_position=(0, 0))
                    nc.tensor.matmul(oph[32:56, 0:C], w3s[:, 24:48],
                                     h2Bsb[:, 0:C], start=True, stop=True,
                                     tile_position=(0, 32))
                    nc.tensor.matmul(oph[64:88, 0:C], w3s[:, 0:24],
                                     h2Asb[:, C:2 * C], start=True, stop=True,
                                     tile_position=(0, 64))
                    nc.tensor.matmul(oph[96:120, 0:C], w3s[:, 24:48],
                                     h2Bsb[:, C:2 * C], start=True, stop=True,
                                     tile_position=(0, 96))
                    evac(outsb[0:120, p * C:p * C + C], oph[0:120, 0:C], C,
                         bias=b3v[0:120, 0:1], relu=False)
                nc.sync.dma_start(
                    out=y[0:120, goff // 2:goff // 2 + GROUP // 2],
                    in_=outsb[0:120, :])

        if iters == 1:
            body()
        else:
            with tc.For_i(0, iters, 1):
                body()
    nc.finalize()
    return nc


def prep_weights(W1, b1, W2, b2, W3, b3):
    W1 = np.asarray(W1, np.float32); b1 = np.asarray(b1, np.float32)
    W2 = np.asarray(W2, np.float32); b2 = np.asarray(b2, np.float32)
    W3 = np.asarray(W3, np.float32); b3 = np.asarray(b3, np.float32)
    # w1q: stripe s at partitions 32s..32s+3 holds half (s%2) of the stacked
    # per-field L1 weights ([x0 x1 x2 1] -> 8 fields x 16 hidden).
    w1s = np.zeros((2, 4, 128), np.float32)
    for half in range(2):
        fb = 8 * half
        for fl in range(8):
            for h in range(HID):
                w1s[half, 0:3, 16 * fl + h] = W1[fb + fl, h, :]
                w1s[half, 3, 16 * fl + h] = b1[fb + fl, h]
    w1q = np.zeros((128, 128), np.float32)
    for s in range(4):
        w1q[32 * s:32 * s + 4, :] = w1s[s % 2]
    w2s = np.zeros((128, 256), np.float32)
    for half in range(2):
        fb = 8 * half
        for fl in range(8):
            blk = W2[fb + fl]  # [g2, h]
            w2s[16 * fl:16 * fl + 16,
                128 * half + 16 * fl:128 * half + 16 * fl + 16] = blk.T
    w3s = np.zeros((128, 48), np.float32)
    for half in range(2):
        fb = 8 * half
        for fl in range(8):
            blk = W3[fb + fl]  # [o, h]
            w3s[16 * fl:16 * fl + 16,
                24 * half + 3 * fl:24 * half + 3 * fl + 3] = blk.T
    b2v = np.zeros((128, 2), np.float32)
    for half in range(2):
        b2v[:, half] = b2[8 * half:8 * half + 8].reshape(128)
    b3v = np.zeros((128, 1), np.float32)
    for q in range(4):
        fb = 8 * (q % 2)
        b3v[32 * q:32 * q + 24, 0] = b3[fb:fb + 8].reshape(24)
    bf = ml_dtypes.bfloat16
    return {
        "w1q": w1q.astype(bf), "w2s": w2s.astype(bf), "w3s": w3s.astype(bf),
        "b2v": b2v, "b3v": b3v,
    }


def _get_nc(n_pad, iters=1):
    key = (n_pad, iters)
    if key not in _cache:
        _cache[key] = build(n_pad, iters)
    return _cache[key]


def run(x_np, weights, n_pad, iters=1, n=None):
    """x_np: [3, N] f32 full; returns [16, 3, N] f32."""
    if n is None:
        n = x_np.shape[1]
    assert n % N_CORES == 0
    npc = n // N_CORES
    assert npc <= n_pad
    nc = _get_nc(n_pad, iters)
    bf = ml_dtypes.bfloat16
    in_maps = []
    for c in range(N_CORES):
        xs = np.zeros((4, n_pad), np.float32)
        xs[0:3, :npc] = x_np[:, c * npc:(c + 1) * npc]
        xs[3, :] = 1.0
        in_maps.append({"xq": xs.astype(bf), **weights})
    res = run_bass_kernel_spmd(nc, in_maps, core_ids=list(range(N_CORES)))
    out = np.empty((NFIELDS, 3, n), np.float32)
    npr = n_pad // PAIR
    for c in range(N_CORES):
        yc = np.asarray(res.results[c]["y"], np.float32)  # [128, n_pad//2]
        yv = yc.reshape(128, npr, C)
        oc = np.empty((NFIELDS, 3, npr, 2, C), np.float32)
        for q in range(4):
            blk = yv[32 * q:32 * q + 24].reshape(8, 3, npr, C)
            oc[8 * (q % 2):8 * (q % 2) + 8, :, :, q // 2, :] = blk
        out[:, :, c * npc:(c + 1) * npc] = \
            oc.reshape(NFIELDS, 3, n_pad)[:, :, :npc]
    return out


def kernel(x, W1, b1, W2, b2, W3, b3, D):
    x = np.asarray(x, np.float32)
    n = x.shape[2]
    npc = n // N_CORES
    n_pad = ((npc + GROUP - 1) // GROUP) * GROUP
    weights = prep_weights(W1, b1, W2, b2, W3, b3)
    return run(x[0], weights, n_pad)


# revision 4
# speedup vs baseline: 1.3953x; 1.3472x over previous
"""Trainium2 Bass kernel for nn_Fields: 16 per-field MLPs (3->16->16->3, ReLU)
applied to 1M points, sharded over 8 NeuronCores along the point axis.

Dataflow per core (points sharded N/8, padded to 4096; slabs of 8192),
processed pair-of-chunks (1024 pts) at a time in three layer phases:
  L1: 2-way ROW-TILED concurrent matmuls (K=4 at row groups 0/32, x
      replicated at partition stripes 0-3 and 32-35) -> one PE span per
      chunk computes both halves of h1 (silicon: tiled MMs overlap).
  L2: block-diag [128x128] per field-half, split as 2 col-tiles of M=64
      (concurrent streams, smaller weight loads).
  L3: 4-way COL-TILED (M=24 at col groups 0/32/64/96) into one PSUM bank.
Evacuation is the wall: DVE and ACT both run ~1 elem/cyc/lane from fp32
PSUM (2x DVE modes need 2-byte sources).  Loads are balanced with
silicon-measured costs; ACT takes the h2 evacs (vector bias is free there).
PSUM (8 banks) is split into two 2-buf pools (h1, h2); the L3 output bank
is a third alloc that alternates between the pools so every cross-pair
reuse edge has ~a pair of slack.
b1 is folded into L1 via a ones row in x. Output rides a quarter-structured
[128, n_pad/2] bf16 DRAM tensor; the host upcasts and unscrambles.
"""

from contextlib import ExitStack

import ml_dtypes
import numpy as np

import concourse.bass as bass
import concourse.mybir as mybir
import concourse.tile as tile
from concourse import bacc
from concourse.bass_utils import run_bass_kernel_spmd

N_CORES = 8
NFIELDS = 16
HID = 16
C = 512  # chunk = one fp32 PSUM bank of matmul output
PAIR = 2 * C  # 1024 points
GROUP = 4096  # padding unit
SLAB = 8192  # DMA slab (2 groups)

BF16 = mybir.dt.bfloat16
F32 = mybir.dt.float32
AF = mybir.ActivationFunctionType
ALU = mybir.AluOpType

_cache = {}


def build(n_pad, iters=1):
    """Build the per-core Bass program for n_pad points (multiple of GROUP)."""
    assert n_pad % GROUP == 0
    slab_sizes = []
    off = 0
    while off < n_pad:
        s = min(SLAB, n_pad - off)
        slab_sizes.append(s)
        off += s

    nc = bacc.Bacc(None, target_bir_lowering=False)
    xq = nc.declare_dram_parameter("xq", [4, n_pad], BF16, isOutput=False)
    w1q_d = nc.declare_dram_parameter("w1q", [64, 128], BF16, isOutput=False)
    w2s_d = nc.declare_dram_parameter("w2s", [128, 256], BF16, isOutput=False)
    w3s_d = nc.declare_dram_parameter("w3s", [128, 48], BF16, isOutput=False)
    b2v_d = nc.declare_dram_parameter("b2v", [128, 2], F32, isOutput=False)
    b3v_d = nc.declare_dram_parameter("b3v", [128, 1], F32, isOutput=False)
    y = nc.declare_dram_parameter("y", [128, n_pad // 2], BF16, isOutput=True)

    with ExitStack() as ctx:
        tc = ctx.enter_context(tile.TileContext(nc))
        consts = ctx.enter_context(tc.tile_pool(name="consts", bufs=1))
        xpool = ctx.enter_context(tc.tile_pool(name="xpool", bufs=2))
        h1pool = ctx.enter_context(tc.tile_pool(name="h1pool", bufs=8))
        h2pool = ctx.enter_context(tc.tile_pool(name="h2pool", bufs=8))
        opool = ctx.enter_context(tc.tile_pool(name="opool", bufs=2))
        psh1 = ctx.enter_context(tc.tile_pool(name="psh1", bufs=2, space="PSUM"))
        psh2 = ctx.enter_context(tc.tile_pool(name="psh2", bufs=2, space="PSUM"))

        w1q = consts.tile([64, 128], BF16)
        nc.sync.dma_start(out=w1q, in_=w1q_d[:, :])
        w2s = consts.tile([128, 256], BF16)
        nc.sync.dma_start(out=w2s, in_=w2s_d[:, :])
        w3s = consts.tile([128, 48], BF16)
        nc.sync.dma_start(out=w3s, in_=w3s_d[:, :])
        b2v = consts.tile([128, 2], F32)
        nc.sync.dma_start(out=b2v, in_=b2v_d[:, :])
        b3v = consts.tile([128, 1], F32)
        nc.sync.dma_start(out=b3v, in_=b3v_d[:, :])

        # Greedy engine balancing with silicon-measured evac costs (ns):
        #   ACT: 288 + 1.086*fd (per-partition bias vector costs nothing)
        #   DVE: 125 + 1.12*fd (1-ALU) / 125 + 1.28*fd (2-ALU with bias)
        load = {"act": 0.0, "dve": 0.0}

        def evac(out_ap, in_ap, fd, bias=None, relu=True):
            cost_act = 288 + 1.086 * fd
            cost_dve = 125 + (1.28 if bias is not None else 1.12) * fd
            if load["act"] + cost_act <= load["dve"] + cost_dve:
                load["act"] += cost_act
                nc.scalar.activation(out_ap, in_ap,
                                     AF.Relu if relu else AF.Identity,
                                     bias=0.0 if bias is None else bias)
            else:
                load["dve"] += cost_dve
                if relu:
                    if bias is None:
                        nc.vector.tensor_scalar_max(out_ap, in_ap, 0.0)
                    else:
                        nc.vector.tensor_scalar(out_ap, in_ap, bias, 0.0,
                                                ALU.add, ALU.max)
                else:
                    if bias is None:
                        nc.vector.tensor_copy(out_ap, in_ap)
                    else:
                        nc.vector.tensor_scalar_add(out_ap, in_ap, bias)

        def body(_=None):
            # global pair list: (slab, pair-in-slab); L3+y run one pair
            # behind L1/L2 so the y evac never head-of-line-blocks the
            # next pair's h1 evacs on the DVE queue.
            outsb_of = {}
            slab_last_pair = {}
            pend = []  # (h2sb pair tiles, slab, p)

            def do_l3(ph2sb, pg, pslab, pp, parity):
                oph = (psh1 if parity == 0 else psh2).tile(
                    [128, PAIR], F32,
                    tag="h1ps" if parity == 0 else "h2ps",
                    name=f"oph_{pg}_{pp}")
                nc.tensor.matmul(oph[0:24, 0:C], w3s[:, 0:24],
                                 ph2sb[0][:, 0:C], start=True, stop=True,
                                 tile_position=(0, 0))
                nc.tensor.matmul(oph[32:56, 0:C], w3s[:, 24:48],
                                 ph2sb[1][:, 0:C], start=True, stop=True,
                                 tile_position=(0, 32))
                nc.tensor.matmul(oph[64:88, 0:C], w3s[:, 0:24],
                                 ph2sb[0][:, C:2 * C], start=True,
                                 stop=True, tile_position=(0, 64))
                nc.tensor.matmul(oph[96:120, 0:C], w3s[:, 24:48],
                                 ph2sb[1][:, C:2 * C], start=True,
                                 stop=True, tile_position=(0, 96))
                osb = outsb_of[pg]
                evac(osb[0:120, pp * C:pp * C + C], oph[0:120, 0:C], C,
                     bias=b3v[0:120, 0:1], relu=False)
                if pp == slab_last_pair[pg]:
                    goff_g, gsz_g = pslab
                    nc.sync.dma_start(
                        out=y[0:120, goff_g // 2:goff_g // 2 + gsz_g // 2],
                        in_=osb[0:120, 0:gsz_g // 2])

            goff = 0
            pglobal = 0
            for g, gsz in enumerate(slab_sizes):
                xsb = xpool.tile([36, SLAB], BF16, tag="xsb", name=f"xsb_{g}")
                for s in range(2):
                    nc.gpsimd.dma_start(out=xsb[32 * s:32 * s + 4, 0:gsz],
                                        in_=xq[0:4, goff:goff + gsz])

                outsb_of[g] = opool.tile([128, SLAB // 2], BF16, tag="outsb",
                                         name=f"outsb_{g}")
                slab_last_pair[g] = gsz // PAIR - 1
                for p in range(gsz // PAIR):
                    c0 = p * PAIR
                    c1 = c0 + C
                    # ---- L1: per chunk, 2 concurrent row-tiled matmuls ----
                    h1sb = h1pool.tile([128, 2 * PAIR], BF16, tag="h1sb",
                                       name=f"h1sb_{g}_{p}")
                    for ci, cc in enumerate((c0, c1)):
                        hp = psh1.tile([128, PAIR], F32, tag="h1ps",
                                       name=f"h1ps_{g}_{p}_{ci}")
                        for s in range(2):
                            nc.tensor.matmul(
                                hp[:, s * C:s * C + C],
                                w1q[32 * s:32 * s + 4, 0:128],
                                xsb[32 * s:32 * s + 4, cc:cc + C],
                                start=True, stop=True,
                                tile_position=(32 * s, 0))
                        evac(h1sb[:, ci * PAIR:(ci + 1) * PAIR], hp, PAIR)

                    # ---- L2: per half, 2 chunks x 2 col-tiles of M=64 ----
                    h2sb = []
                    for hf in range(2):
                        hp = psh2.tile([128, PAIR], F32, tag="h2ps",
                                       name=f"h2ps_{g}_{p}_{hf}")
                        for ci in range(2):
                            rhs = h1sb[:, ci * PAIR + hf * C:
                                       ci * PAIR + hf * C + C]
                            for j in range(2):
                                nc.tensor.matmul(
                                    hp[64 * j:64 * j + 64, ci * C:ci * C + C],
                                    w2s[:, 128 * hf + 64 * j:
                                        128 * hf + 64 * j + 64], rhs,
                                    start=True, stop=True,
                                    tile_position=(0, 64 * j))
                        t = h2pool.tile([128, PAIR], BF16, tag="h2sb",
                                        name=f"h2sb_{g}_{p}_{hf}")
                        evac(t, hp, PAIR, bias=b2v[:, hf:hf + 1])
                        h2sb.append(t)

                    # ---- L3 for the PREVIOUS pair (software pipeline) ----
                    pend.append((h2sb, g, (goff, gsz), p))
                    if len(pend) > 1:
                        ph2sb, pg, pslab, pp = pend.pop(0)
                        do_l3(ph2sb, pg, pslab, pp, pglobal % 2)
                    pglobal += 1
                goff += gsz
            ph2sb, pg, pslab, pp = pend.pop(0)
            do_l3(ph2sb, pg, pslab, pp, pglobal % 2)

        if iters == 1:
            body()
        else:
            with tc.For_i(0, iters, 1):
                body()
    nc.finalize()
    return nc


def prep_weights(W1, b1, W2, b2, W3, b3):
    W1 = np.asarray(W1, np.float32); b1 = np.asarray(b1, np.float32)
    W2 = np.asarray(W2, np.float32); b2 = np.asarray(b2, np.float32)
    W3 = np.asarray(W3, np.float32); b3 = np.asarray(b3, np.float32)
    # w1q: stripe s at partitions 32s..32s+3 holds half s of the stacked
    # per-field L1 weights ([x0 x1 x2 1] -> 8 fields x 16 hidden).
    w1q = np.zeros((64, 128), np.float32)
    for half in range(2):
        fb = 8 * half
        for fl in range(8):
            for h in range(HID):
                w1q[32 * half:32 * half + 3, 16 * fl + h] = W1[fb + fl, h, :]
                w1q[32 * half + 3, 16 * fl + h] = b1[fb + fl, h]
    w2s = np.zeros((128, 256), np.float32)
    for half in range(2):
        fb = 8 * half
        for fl in range(8):
            blk = W2[fb + fl]  # [g2, h]
            w2s[16 * fl:16 * fl + 16,
                128 * half + 16 * fl:128 * half + 16 * fl + 16] = blk.T
    w3s = np.zeros((128, 48), np.float32)
    for half in range(2):
        fb = 8 * half
        for fl in range(8):
            blk = W3[fb + fl]  # [o, h]
            w3s[16 * fl:16 * fl + 16,
                24 * half + 3 * fl:24 * half + 3 * fl + 3] = blk.T
    b2v = np.zeros((128, 2), np.float32)
    for half in range(2):
        b2v[:, half] = b2[8 * half:8 * half + 8].reshape(128)
    b3v = np.zeros((128, 1), np.float32)
    for q in range(4):
        fb = 8 * (q % 2)
        b3v[32 * q:32 * q + 24, 0] = b3[fb:fb + 8].reshape(24)
    bf = ml_dtypes.bfloat16
    return {
        "w1q": w1q.astype(bf), "w2s": w2s.astype(bf), "w3s": w3s.astype(bf),
        "b2v": b2v, "b3v": b3v,
    }


def _get_nc(n_pad, iters=1):
    key = (n_pad, iters)
    if key not in _cache:
        _cache[key] = build(n_pad, iters)
    return _cache[key]


def run(x_np, weights, n_pad, iters=1, n=None):
    """x_np: [3, N] f32 full; returns [16, 3, N] f32."""
    if n is None:
        n = x_np.shape[1]
    assert n % N_CORES == 0
    npc = n // N_CORES
    assert npc <= n_pad
    nc = _get_nc(n_pad, iters)
    bf = ml_dtypes.bfloat16
    in_maps = []
    for c in range(N_CORES):
        xs = np.zeros((4, n_pad), np.float32)
        xs[0:3, :npc] = x_np[:, c * npc:(c + 1) * npc]
        xs[3, :] = 1.0
        in_maps.append({"xq": xs.astype(bf), **weights})
    res = run_bass_kernel_spmd(nc, in_maps, core_ids=list(range(N_CORES)))
    out = np.empty((NFIELDS, 3, n), np.float32)
    npr = n_pad // PAIR
    for c in range(N_CORES):
        yc = np.asarray(res.results[c]["y"], np.float32)  # [128, n_pad//2]
        yv = yc.reshape(128, npr, C)
        oc = np.empty((NFIELDS, 3, npr, 2, C), np.float32)
        for q in range(4):
            blk = yv[32 * q:32 * q + 24].reshape(8, 3, npr, C)
            oc[8 * (q % 2):8 * (q % 2) + 8, :, :, q // 2, :] = blk
        out[:, :, c * npc:(c + 1) * npc] = \
            oc.reshape(NFIELDS, 3, n_pad)[:, :, :npc]
    return out


def kernel(x, W1, b1, W2, b2, W3, b3, D):
    x = np.asarray(x, np.float32)
    n = x.shape[2]
    npc = n // N_CORES
    n_pad = ((npc + GROUP - 1) // GROUP) * GROUP
    weights = prep_weights(W1, b1, W2, b2, W3, b3)
    return run(x[0], weights, n_pad)


# revision 6
# speedup vs baseline: 1.9265x; 1.3807x over previous
"""Trainium2 Bass kernel for nn_Fields: 16 per-field MLPs (3->16->16->3, ReLU)
applied to 1M points, sharded over 8 NeuronCores along the point axis.

Dataflow per core (points sharded N/8, padded to 4096; slabs of 8192),
processed pair-of-chunks (1024 pts) at a time in three layer phases:
  L1: 2-way ROW-TILED concurrent matmuls (K=4 at row groups 0/32, x
      replicated at partition stripes 0-3 and 32-35) -> one PE span per
      chunk computes both halves of h1 (silicon: tiled MMs overlap).
  L2: block-diag [128x128] per field-half, split as 2 col-tiles of M=64
      (concurrent streams, smaller weight loads).
  L3: 4-way COL-TILED (M=24 at col groups 0/32/64/96) into one PSUM bank.
Evacuation is the wall: DVE and ACT both run ~1 elem/cyc/lane from fp32
PSUM (2x DVE modes need 2-byte sources).  Loads are balanced with
silicon-measured costs; ACT takes the h2 evacs (vector bias is free there).
PSUM (8 banks) is split into two 2-buf pools (h1, h2); the L3 output bank
is a third alloc that alternates between the pools so every cross-pair
reuse edge has ~a pair of slack.
b1 is folded into L1 via a ones row in x. Output rides a quarter-structured
[128, n_pad/2] bf16 DRAM tensor; the host upcasts and unscrambles.
"""

from contextlib import ExitStack

import ml_dtypes
import numpy as np

import concourse.bass as bass
import concourse.mybir as mybir
import concourse.tile as tile
from concourse import bacc
from concourse.bass_utils import run_bass_kernel_spmd

N_CORES = 8
NFIELDS = 16
HID = 16
C = 512  # chunk = one fp32 PSUM bank of matmul output
PAIR = 2 * C  # 1024 points
GROUP = 4096  # padding unit
SLAB = 8192  # DMA slab (2 groups)

BF16 = mybir.dt.bfloat16
F32 = mybir.dt.float32
AF = mybir.ActivationFunctionType
ALU = mybir.AluOpType

_cache = {}


def build(n_pad, iters=1):
    """Build the per-core Bass program for n_pad points (multiple of GROUP)."""
    assert n_pad % GROUP == 0
    slab_sizes = []
    off = 0
    while off < n_pad:
        s = min(SLAB, n_pad - off)
        slab_sizes.append(s)
        off += s

    nc = bacc.Bacc(None, target_bir_lowering=False)
    xq = nc.declare_dram_parameter("xq", [4, n_pad], BF16, isOutput=False)
    w1q_d = nc.declare_dram_parameter("w1q", [64, 128], BF16, isOutput=False)
    w2s_d = nc.declare_dram_parameter("w2s", [128, 256], BF16, isOutput=False)
    w3s_d = nc.declare_dram_parameter("w3s", [128, 48], BF16, isOutput=False)
    b2v_d = nc.declare_dram_parameter("b2v", [128, 2], F32, isOutput=False)
    b3v_d = nc.declare_dram_parameter("b3v", [128, 1], F32, isOutput=False)
    y = nc.declare_dram_parameter("y", [128, n_pad // 2], BF16, isOutput=True)

    with ExitStack() as ctx:
        tc = ctx.enter_context(tile.TileContext(nc))
        consts = ctx.enter_context(tc.tile_pool(name="consts", bufs=1))
        xpool = ctx.enter_context(tc.tile_pool(name="xpool", bufs=2))
        h1pool = ctx.enter_context(tc.tile_pool(name="h1pool", bufs=18))
        h2pool = ctx.enter_context(tc.tile_pool(name="h2pool", bufs=18))
        opool = ctx.enter_context(tc.tile_pool(name="opool", bufs=2))
        psh1 = ctx.enter_context(tc.tile_pool(name="psh1", bufs=4, space="PSUM"))

        w1q = consts.tile([64, 128], BF16)
        nc.sync.dma_start(out=w1q, in_=w1q_d[:, :])
        w2s = consts.tile([128, 256], BF16)
        nc.sync.dma_start(out=w2s, in_=w2s_d[:, :])
        w3s = consts.tile([128, 48], BF16)
        nc.sync.dma_start(out=w3s, in_=w3s_d[:, :])
        b2v = consts.tile([128, 2], F32)
        nc.sync.dma_start(out=b2v, in_=b2v_d[:, :])
        b3v = consts.tile([128, 1], F32)
        nc.sync.dma_start(out=b3v, in_=b3v_d[:, :])

        # Greedy engine balancing with silicon-measured evac costs (ns):
        #   ACT: 288 + 1.086*fd (per-partition bias vector costs nothing)
        #   DVE: 125 + 1.12*fd (1-ALU) / 125 + 1.28*fd (2-ALU with bias)
        load = {"act": 0.0, "dve": 0.0}

        def evac(out_ap, in_ap, fd, bias=None, relu=True):
            cost_act = 288 + 1.086 * fd
            cost_dve = 125 + (1.28 if bias is not None else 1.12) * fd
            if load["act"] + cost_act <= load["dve"] + cost_dve:
                load["act"] += cost_act
                nc.scalar.activation(out_ap, in_ap,
                                     AF.Relu if relu else AF.Identity,
                                     bias=0.0 if bias is None else bias)
            else:
                load["dve"] += cost_dve
                if relu:
                    if bias is None:
                        nc.vector.tensor_scalar_max(out_ap, in_ap, 0.0)
                    else:
                        nc.vector.tensor_scalar(out_ap, in_ap, bias, 0.0,
                                                ALU.add, ALU.max)
                else:
                    if bias is None:
                        nc.vector.tensor_copy(out_ap, in_ap)
                    else:
                        nc.vector.tensor_scalar_add(out_ap, in_ap, bias)

        def body(_=None):
            goff = 0
            for g, gsz in enumerate(slab_sizes):
                nch = gsz // C
                npairs = gsz // PAIR
                xsb = xpool.tile([36, SLAB], BF16, tag="xsb", name=f"xsb_{g}")
                for s in range(2):
                    nc.gpsimd.dma_start(out=xsb[32 * s:32 * s + 4, 0:gsz],
                                        in_=xq[0:4, goff:goff + gsz])
                outsb = opool.tile([128, SLAB // 2], BF16, tag="outsb",
                                   name=f"outsb_{g}")

                # ---- phase L1: all chunks, 2 concurrent row-tiled MMs ----
                h1sb = []
                for c in range(nch):
                    hp = psh1.tile([128, PAIR], F32, tag="hps",
                                   name=f"h1ps_{g}_{c}")
                    for s in range(2):
                        nc.tensor.matmul(
                            hp[:, s * C:s * C + C],
                            w1q[32 * s:32 * s + 4, 0:128],
                            xsb[32 * s:32 * s + 4, c * C:c * C + C],
                            start=True, stop=True,
                            tile_position=(32 * s, 0))
                    t = h1pool.tile([128, PAIR], BF16, tag="h1sb",
                                    name=f"h1sb_{g}_{c}")
                    evac(t, hp, PAIR)
                    h1sb.append(t)

                # ---- phase L2: all half-a, then all half-b (col2 tiles) ----
                h2sb = {}
                for hf in range(2):
                    hps = []
                    for p in range(npairs):
                        hp = psh1.tile([128, PAIR], F32, tag="hps",
                                       name=f"h2ps_{g}_{p}_{hf}")
                        for ci in range(2):
                            rhs = h1sb[2 * p + ci][:, hf * C:hf * C + C]
                            for j in range(2):
                                nc.tensor.matmul(
                                    hp[64 * j:64 * j + 64, ci * C:ci * C + C],
                                    w2s[:, 128 * hf + 64 * j:
                                        128 * hf + 64 * j + 64], rhs,
                                    start=True, stop=True,
                                    tile_position=(0, 64 * j))
                        hps.append(hp)
                    for p in range(npairs):
                        t = h2pool.tile([128, PAIR], BF16, tag="h2sb",
                                        name=f"h2sb_{g}_{p}_{hf}")
                        evac(t, hps[p], PAIR, bias=b2v[:, hf:hf + 1])
                        h2sb[(p, hf)] = t

                # ---- phase L3: per pair, 4 col-tiled MMs + y evac ----
                for p in range(npairs):
                    oph = psh1.tile([128, PAIR], F32, tag="hps",
                                    name=f"oph_{g}_{p}")
                    a, b = h2sb[(p, 0)], h2sb[(p, 1)]
                    nc.tensor.matmul(oph[0:24, 0:C], w3s[:, 0:24],
                                     a[:, 0:C], start=True, stop=True,
                                     tile_position=(0, 0))
                    nc.tensor.matmul(oph[32:56, 0:C], w3s[:, 24:48],
                                     b[:, 0:C], start=True, stop=True,
                                     tile_position=(0, 32))
                    nc.tensor.matmul(oph[64:88, 0:C], w3s[:, 0:24],
                                     a[:, C:2 * C], start=True, stop=True,
                                     tile_position=(0, 64))
                    nc.tensor.matmul(oph[96:120, 0:C], w3s[:, 24:48],
                                     b[:, C:2 * C], start=True, stop=True,
                                     tile_position=(0, 96))
                    evac(outsb[0:120, p * C:p * C + C], oph[0:120, 0:C], C,
                         bias=b3v[0:120, 0:1], relu=False)
                nc.sync.dma_start(
                    out=y[0:120, goff // 2:goff // 2 + gsz // 2],
                    in_=outsb[0:120, 0:gsz // 2])
                goff += gsz

        if iters == 1:
            body()
        else:
            with tc.For_i(0, iters, 1):
                body()
    nc.finalize()
    return nc


def prep_weights(W1, b1, W2, b2, W3, b3):
    W1 = np.asarray(W1, np.float32); b1 = np.asarray(b1, np.float32)
    W2 = np.asarray(W2, np.float32); b2 = np.asarray(b2, np.float32)
    W3 = np.asarray(W3, np.float32); b3 = np.asarray(b3, np.float32)
    # w1q: stripe s at partitions 32s..32s+3 holds half s of the stacked
    # per-field L1 weights ([x0 x1 x2 1] -> 8 fields x 16 hidden).
    w1q = np.zeros((64, 128), np.float32)
    for half in range(2):
        fb = 8 * half
        for fl in range(8):
            for h in range(HID):
                w1q[32 * half:32 * half + 3, 16 * fl + h] = W1[fb + fl, h, :]
                w1q[32 * half + 3, 16 * fl + h] = b1[fb + fl, h]
    w2s = np.zeros((128, 256), np.float32)
    for half in range(2):
        fb = 8 * half
        for fl in range(8):
            blk = W2[fb + fl]  # [g2, h]
            w2s[16 * fl:16 * fl + 16,
                128 * half + 16 * fl:128 * half + 16 * fl + 16] = blk.T
    w3s = np.zeros((128, 48), np.float32)
    for half in range(2):
        fb = 8 * half
        for fl in range(8):
            blk = W3[fb + fl]  # [o, h]
            w3s[16 * fl:16 * fl + 16,
                24 * half + 3 * fl:24 * half + 3 * fl + 3] = blk.T
    b2v = np.zeros((128, 2), np.float32)
    for half in range(2):
        b2v[:, half] = b2[8 * half:8 * half + 8].reshape(128)
    b3v = np.zeros((128, 1), np.float32)
    for q in range(4):
        fb = 8 * (q % 2)
        b3v[32 * q:32 * q + 24, 0] = b3[fb:fb + 8].reshape(24)
    bf = ml_dtypes.bfloat16
    return {
        "w1q": w1q.astype(bf), "w2s": w2s.astype(bf), "w3s": w3s.astype(bf),
        "b2v": b2v, "b3v": b3v,
    }


def _get_nc(n_pad, iters=1):
    key = (n_pad, iters)
    if key not in _cache:
        _cache[key] = build(n_pad, iters)
    return _cache[key]


def run(x_np, weights, n_pad, iters=1, n=None):
    """x_np: [3, N] f32 full; returns [16, 3, N] f32."""
    if n is None:
        n = x_np.shape[1]
    assert n % N_CORES == 0
    npc = n // N_CORES
    assert npc <= n_pad
    nc = _get_nc(n_pad, iters)
    bf = ml_dtypes.bfloat16
    in_maps = []
    for c in range(N_CORES):
        xs = np.zeros((4, n_pad), np.float32)
        xs[0:3, :npc] = x_np[:, c * npc:(c + 1) * npc]
        xs[3, :] = 1.0
        in_maps.append({"xq": xs.astype(bf), **weights})
    res = run_bass_kernel_spmd(nc, in_maps, core_ids=list(range(N_CORES)))
    out = np.empty((NFIELDS, 3, n), np.float32)
    npr = n_pad // PAIR
    for c in range(N_CORES):
        yc = np.asarray(res.results[c]["y"], np.float32)  # [128, n_pad//2]
        yv = yc.reshape(128, npr, C)
        oc = np.empty((NFIELDS, 3, npr, 2, C), np.float32)
        for q in range(4):
            blk = yv[32 * q:32 * q + 24].reshape(8, 3, npr, C)
            oc[8 * (q % 2):8 * (q % 2) + 8, :, :, q // 2, :] = blk
        out[:, :, c * npc:(c + 1) * npc] = \
            oc.reshape(NFIELDS, 3, n_pad)[:, :, :npc]
    return out


def kernel(x, W1, b1, W2, b2, W3, b3, D):
    x = np.asarray(x, np.float32)
    n = x.shape[2]
    npc = n // N_CORES
    n_pad = ((npc + GROUP - 1) // GROUP) * GROUP
    weights = prep_weights(W1, b1, W2, b2, W3, b3)
    return run(x[0], weights, n_pad)
